# revision 4
# baseline (speedup 1.0000x reference)
"""Expected Calibration Error kernel for Trainium2 (Bass/Tile), 8 NeuronCores, v4.

Problem: logits [1000000, 100] f32, labels [1000000] i64 ->
  (ece [1] f32, acc [1] f32)

v4 strategy (v3 + shard rebalance + DMA descriptor-size tuning):
  - ROWS_PART=980 rows/partition/core (125440 rows/core, the closest
    tileable value to N/8/128 = 976.56): 1.2% less per-core HBM traffic
    than v3's 992.
  - R rows/partition per tile, T = 980 // R tiles/core. Each tile's logits
    slab ([128, R, 100] f32) is one contiguous HBM->SBUF DMA (R*400 bytes
    per partition line), alternating between the two HWDGE queues
    (sync/scalar) so both are always busy; the per-core DMA path
    (~360-420 GB/s effective) is the roofline for this kernel.
  - Per-tile DVE: grouped reduce_max -> conf; is_equal(chosen, conf) -> acc;
    is_gt(conf, bounds) -> cumulative bin masks G [128, R, 15].
  - v tiles hold interleaved triples (conf, acc, 1) per row-slot; the ones
    lane is written once at kernel start (tiles are pre-allocated per t and
    reused across reps).
  - Per-tile matmul chunks (row-slot splits of R with cs*15 <= 512 each):
    psum_ck [3*cs, cs*15] += v[:, a:b, :]^T @ G[:, a:b, :], accumulated
    across tiles and reps.
  - Host folds the per-core psum chunks (diagonal r-slot blocks), differences
    cumulative bounds into 15 bins, applies the ECE formula.
"""

import numpy as np

P = 128          # SBUF partitions
C = 100          # classes
NCORES = 8
NB = 15          # bin boundaries used on-device (bound 15 = conf>1.0 == 0 always)
ROWS_PART = 980  # rows per partition per core (980*128*8 = 1,003,520 >= N)
ROWS_CORE = P * ROWS_PART      # 125440
N = 1_000_000

R = 49           # rows per partition per tile (19.6KB DMA descriptors)
T = ROWS_PART // R
CHUNKS = [25, 24]  # row-slot split per tile; each cs*NB <= 512 (PSUM bank)
SMAX = max(CHUNKS)
STARTS = np.cumsum([0] + CHUNKS).tolist()
XBUFS = 4

_CACHE = {}


def _build_nc(reps=1, xbufs=XBUFS):
    import concourse.bass as bass
    import concourse.bacc as bacc
    import concourse.mybir as mybir
    import concourse.tile as tile

    f32 = mybir.dt.float32
    nc = bacc.Bacc()

    logits_d = nc.dram_tensor("logits", [ROWS_CORE, C], f32, kind="ExternalInput")
    chosen_d = nc.dram_tensor("chosen", [P, T * R], f32, kind="ExternalInput")
    bounds_d = nc.dram_tensor("bounds", [1, NB], f32, kind="ExternalInput")
    stats_d = nc.dram_tensor("stats", [3 * SMAX, len(CHUNKS) * SMAX * NB], f32,
                             kind="ExternalOutput")

    # [T, 128, R*C] view of the row-major logits: partition p of tile t holds
    # rows (t*128 + p)*R ... + R.
    lx = logits_d[:].flatten().rearrange("(t p f) -> t p f", t=T, p=P, f=R * C)

    with tile.TileContext(nc) as tc:
        with (
            tc.tile_pool(name="singles", bufs=1) as singles,
            tc.tile_pool(name="xtiles", bufs=xbufs) as xtiles,
            tc.tile_pool(name="vals", bufs=1) as valsp,
            tc.tile_pool(name="gmask", bufs=4) as gmaskp,
            tc.tile_pool(name="psum", bufs=1, space="PSUM") as psump,
        ):
            bounds_sb = singles.tile([P, NB], f32)
            nc.sync.dma_start(
                out=bounds_sb[:],
                in_=bass.AP(tensor=bounds_d, offset=0, ap=[[0, P], [1, NB]]),
            )
            chosen_sb = singles.tile([P, T * R], f32)
            nc.sync.dma_start(out=chosen_sb[:], in_=chosen_d[:])
            # First-touch the DMA'd singles on DVE so in-loop ops never carry
            # a second sync-wait (walrus core_v3 allows 1 wait/instruction).
            touch = singles.tile([P, 2], f32)
            nc.vector.tensor_copy(out=touch[:, 0:1], in_=chosen_sb[:, 0:1])
            nc.vector.tensor_copy(out=touch[:, 1:2], in_=bounds_sb[:, 0:1])

            # One v tile per t, reused across reps; ones lane written once.
            vlist = []
            for t in range(T):
                v = valsp.tile([P, R, 3], f32, tag=f"v{t}", name=f"v{t}")
                nc.vector.memset(v[:, :, 2], 1.0)
                vlist.append(v)

            psums = [psump.tile([3 * cs, cs * NB], f32, tag=f"ps{ck}",
                                name=f"ps{ck}")
                     for ck, cs in enumerate(CHUNKS)]

            bounds_b = bounds_sb[:].unsqueeze(1).broadcast_to([P, R, NB])

            for rep in range(reps):
                for t in range(T):
                    x = xtiles.tile([P, R, C], f32)
                    src = lx[t].rearrange("p (r c) -> p r c", r=R)
                    eng = nc.sync if t % 2 == 0 else nc.scalar
                    eng.dma_start(out=x[:], in_=src)

                    v = vlist[t]
                    conf = v[:, :, 0]
                    nc.vector.tensor_reduce(
                        out=conf, in_=x[:], axis=mybir.AxisListType.X,
                        op=mybir.AluOpType.max,
                    )
                    nc.vector.tensor_tensor(
                        out=v[:, :, 1],
                        in0=chosen_sb[:, t * R:(t + 1) * R],
                        in1=conf,
                        op=mybir.AluOpType.is_equal,
                    )
                    g = gmaskp.tile([P, R, NB], f32)
                    nc.vector.tensor_tensor(
                        out=g[:],
                        in0=conf.unsqueeze(2).broadcast_to([P, R, NB]),
                        in1=bounds_b,
                        op=mybir.AluOpType.is_gt,
                    )
                    first = (rep == 0 and t == 0)
                    last = (rep == reps - 1 and t == T - 1)
                    for ck, cs in enumerate(CHUNKS):
                        a, b = STARTS[ck], STARTS[ck + 1]
                        nc.tensor.matmul(
                            psums[ck][:],
                            v[:, a:b, :].rearrange("p r s -> p (r s)"),
                            g[:, a:b, :].rearrange("p r j -> p (r j)"),
                            start=first, stop=last,
                        )

            out_sb = singles.tile([3 * SMAX, len(CHUNKS) * SMAX * NB], f32)
            nc.vector.memset(out_sb[:], 0.0)
            for ck, cs in enumerate(CHUNKS):
                nc.vector.tensor_copy(
                    out=out_sb[:3 * cs,
                               ck * SMAX * NB:ck * SMAX * NB + cs * NB],
                    in_=psums[ck][:],
                )
            nc.sync.dma_start(out=stats_d[:], in_=out_sb[:])

    nc.finalize()
    return nc


def _get_nc():
    if "nc" not in _CACHE:
        _CACHE["nc"] = _build_nc()
    return _CACHE["nc"]


def _prep_inputs(logits, labels):
    """Shard + host-side prep. Returns in_maps for run_bass_kernel_spmd."""
    logits = np.asarray(logits)
    labels = np.asarray(labels)
    assert logits.shape == (N, C) and logits.dtype == np.float32

    bounds = np.linspace(0.0, 1.0, 16, dtype=np.float32)[:NB]
    chosen = np.take_along_axis(
        logits, labels.reshape(-1, 1).astype(np.int64), axis=1
    ).reshape(-1)

    in_maps = []
    for c in range(NCORES):
        lo = c * ROWS_CORE
        hi = lo + ROWS_CORE
        if hi <= N:
            lg = logits[lo:hi]           # view, no copy
            ch = chosen[lo:hi]
        else:
            npad = hi - N
            lg = np.vstack([logits[lo:N], np.full((npad, C), -1.0, np.float32)])
            ch = np.concatenate([chosen[lo:N], np.zeros(npad, np.float32)])
        ch_t = np.ascontiguousarray(
            ch.reshape(T, P, R).transpose(1, 0, 2).reshape(P, T * R)
        )
        in_maps.append({"logits": lg, "chosen": ch_t, "bounds": bounds.reshape(1, NB)})
    return in_maps


def _finish(outs):
    """Fold per-core [3*SMAX, len(CHUNKS)*SMAX*NB] stats into (ece, acc)."""
    cum_conf = np.zeros(NB, np.float64)
    cum_acc = np.zeros(NB, np.float64)
    cum_cnt = np.zeros(NB, np.float64)
    for o in outs:
        o = np.asarray(o, np.float64)
        for ck, cs in enumerate(CHUNKS):
            off = ck * SMAX * NB
            blk = o[:3 * cs, off:off + cs * NB]
            blk = blk.reshape(cs, 3, cs, NB)     # [r][s][r2][j]
            r_idx = np.arange(cs)
            cum_conf += blk[r_idx, 0, r_idx, :].sum(axis=0)
            cum_acc += blk[r_idx, 1, r_idx, :].sum(axis=0)
            cum_cnt += blk[r_idx, 2, r_idx, :].sum(axis=0)

    count = cum_cnt - np.append(cum_cnt[1:], 0.0)
    sconf = cum_conf - np.append(cum_conf[1:], 0.0)
    sacc = cum_acc - np.append(cum_acc[1:], 0.0)

    safe = count > 0
    denom = np.where(safe, count, 1.0)
    conf_in = sconf / denom
    acc_in = sacc / denom
    prop = count / float(N)
    ece = float(np.where(safe, np.abs(conf_in - acc_in) * prop, 0.0).sum() * 100.0)
    acc = float(np.where(safe, acc_in * prop, 0.0).sum() * 100.0)
    return (
        np.array([ece], np.float32),
        np.array([acc], np.float32),
    )


def _run(logits, labels, trace=False):
    from concourse.bass_utils import run_bass_kernel_spmd

    nc = _get_nc()
    in_maps = _prep_inputs(logits, labels)
    res = run_bass_kernel_spmd(
        nc, in_maps, core_ids=list(range(NCORES)), trace=trace,
    )
    outs = [r["stats"] for r in res.results]
    return _finish(outs), res


def kernel(logits, labels):
    out, _ = _run(logits, labels)
    return out


# revision 6
# speedup vs baseline: 1.0446x; 1.0446x over previous
"""Expected Calibration Error kernel for Trainium2 (Bass/Tile), 8 NeuronCores, v5.

Problem: logits [1000000, 100] f32, labels [1000000] i64 ->
  (ece [1] f32, acc [1] f32)

v5 = v4 (R=98 tiles, 2-queue HWDGE ping-pong, DVE reduce + PE binning
matmuls, at the per-core HBM DMA roofline) + ragged last tile:
ROWS_PART 980 -> 977 (9 tiles of R=98 + 1 tile of R2=95), trimming the
shard to 125,056 rows/core vs the ideal N/8 = 125,000 — 0.31% less DMA
per core than v4's 125,440.

The ragged tile's third row-chunk (29 slots) accumulates into its own
PSUM tile (ps3) so every PSUM accumulation group is opened/closed by
full-region matmuls.
"""

import numpy as np

P = 128          # SBUF partitions
C = 100          # classes
NCORES = 8
NB = 15          # bin boundaries used on-device (bound 15 = conf>1.0 == 0 always)
ROWS_PART = 977  # rows per partition per core (977*128*8 = 1,000,448 >= N)
ROWS_CORE = P * ROWS_PART      # 125056
N = 1_000_000

R = 98           # rows per partition per main tile (39.2KB DMA descriptors)
T1 = 9           # number of main tiles
R2 = 95          # rows per partition in the ragged last tile (9*98+95 = 977)
SMAX = 33
XBUFS = 3
# (cs, psum row-slot source range) per PSUM block; blocks 0,1 are shared by
# main and ragged tiles, block 2 is main-tile rows 66:98, block 3 is
# ragged-tile rows 66:95.
BLOCK_CS = [33, 33, 32, 29]

_CACHE = {}


def _build_nc(reps=1, xbufs=XBUFS):
    import concourse.bass as bass
    import concourse.bacc as bacc
    import concourse.mybir as mybir
    import concourse.tile as tile

    f32 = mybir.dt.float32
    nc = bacc.Bacc()

    logits_d = nc.dram_tensor("logits", [ROWS_CORE, C], f32, kind="ExternalInput")
    chosen_d = nc.dram_tensor("chosen", [P, ROWS_PART], f32, kind="ExternalInput")
    bounds_d = nc.dram_tensor("bounds", [1, NB], f32, kind="ExternalInput")
    stats_d = nc.dram_tensor("stats", [3 * SMAX, 4 * SMAX * NB], f32,
                             kind="ExternalOutput")

    # Main region: partition p of tile t holds rows (t*128 + p)*98 ... + 98.
    # Ragged region: partition p holds rows 112896 + p*95 ... + 95.
    flat = logits_d[:].flatten()
    na = T1 * P * R * C
    lxA = flat[0:na].rearrange("(t p f) -> t p f", t=T1, p=P, f=R * C)
    lxB = flat[na:na + P * R2 * C].rearrange("(p f) -> p f", p=P, f=R2 * C)

    with tile.TileContext(nc) as tc:
        with (
            tc.tile_pool(name="singles", bufs=1) as singles,
            tc.tile_pool(name="xtiles", bufs=xbufs) as xtiles,
            tc.tile_pool(name="xlast", bufs=1) as xlast,
            tc.tile_pool(name="vals", bufs=1) as valsp,
            tc.tile_pool(name="gmask", bufs=4) as gmaskp,
            tc.tile_pool(name="glast", bufs=1) as glastp,
            tc.tile_pool(name="psum", bufs=1, space="PSUM") as psump,
        ):
            bounds_sb = singles.tile([P, NB], f32)
            nc.sync.dma_start(
                out=bounds_sb[:],
                in_=bass.AP(tensor=bounds_d, offset=0, ap=[[0, P], [1, NB]]),
            )
            chosen_sb = singles.tile([P, ROWS_PART], f32)
            nc.sync.dma_start(out=chosen_sb[:], in_=chosen_d[:])
            # First-touch the DMA'd singles on DVE so in-loop ops never carry
            # a second sync-wait (walrus core_v3 allows 1 wait/instruction).
            touch = singles.tile([P, 2], f32)
            nc.vector.tensor_copy(out=touch[:, 0:1], in_=chosen_sb[:, 0:1])
            nc.vector.tensor_copy(out=touch[:, 1:2], in_=bounds_sb[:, 0:1])

            # One v tile per t, reused across reps; ones lane written once.
            vlist = []
            for t in range(T1):
                v = valsp.tile([P, R, 3], f32, tag=f"v{t}", name=f"v{t}")
                nc.vector.memset(v[:, :, 2], 1.0)
                vlist.append(v)
            vB = valsp.tile([P, R2, 3], f32, tag="vB", name="vB")
            nc.vector.memset(vB[:, :, 2], 1.0)

            psums = [psump.tile([3 * cs, cs * NB], f32, tag=f"ps{k}",
                                name=f"ps{k}")
                     for k, cs in enumerate(BLOCK_CS)]

            bounds_bA = bounds_sb[:].unsqueeze(1).broadcast_to([P, R, NB])
            bounds_bB = bounds_sb[:].unsqueeze(1).broadcast_to([P, R2, NB])

            for rep in range(reps):
                for t in range(T1):
                    x = xtiles.tile([P, R, C], f32)
                    src = lxA[t].rearrange("p (r c) -> p r c", r=R)
                    eng = nc.sync if t % 2 == 0 else nc.scalar
                    eng.dma_start(out=x[:], in_=src)

                    v = vlist[t]
                    conf = v[:, :, 0]
                    nc.vector.tensor_reduce(
                        out=conf, in_=x[:], axis=mybir.AxisListType.X,
                        op=mybir.AluOpType.max,
                    )
                    nc.vector.tensor_tensor(
                        out=v[:, :, 1],
                        in0=chosen_sb[:, t * R:(t + 1) * R],
                        in1=conf,
                        op=mybir.AluOpType.is_equal,
                    )
                    g = gmaskp.tile([P, R, NB], f32)
                    nc.vector.tensor_tensor(
                        out=g[:],
                        in0=conf.unsqueeze(2).broadcast_to([P, R, NB]),
                        in1=bounds_bA,
                        op=mybir.AluOpType.is_gt,
                    )
                    first = (rep == 0 and t == 0)
                    # ps0/ps1 close on the ragged tile; ps2 closes here.
                    for ck, (a, b) in enumerate(((0, 33), (33, 66), (66, 98))):
                        stop = (ck == 2 and rep == reps - 1 and t == T1 - 1)
                        nc.tensor.matmul(
                            psums[ck][:],
                            v[:, a:b, :].rearrange("p r s -> p (r s)"),
                            g[:, a:b, :].rearrange("p r j -> p (r j)"),
                            start=first, stop=stop,
                        )

                # Ragged last tile (10th DMA of the rep, scalar queue).
                xb = xlast.tile([P, R2, C], f32)
                nc.scalar.dma_start(
                    out=xb[:], in_=lxB.rearrange("p (r c) -> p r c", r=R2))
                confB = vB[:, :, 0]
                nc.vector.tensor_reduce(
                    out=confB, in_=xb[:], axis=mybir.AxisListType.X,
                    op=mybir.AluOpType.max,
                )
                nc.vector.tensor_tensor(
                    out=vB[:, :, 1],
                    in0=chosen_sb[:, T1 * R:ROWS_PART],
                    in1=confB,
                    op=mybir.AluOpType.is_equal,
                )
                gB = glastp.tile([P, R2, NB], f32)
                nc.vector.tensor_tensor(
                    out=gB[:],
                    in0=confB.unsqueeze(2).broadcast_to([P, R2, NB]),
                    in1=bounds_bB,
                    op=mybir.AluOpType.is_gt,
                )
                last_rep = (rep == reps - 1)
                for ck, (a, b) in [(0, (0, 33)), (1, (33, 66)),
                                   (3, (66, 95))]:
                    nc.tensor.matmul(
                        psums[ck][:],
                        vB[:, a:b, :].rearrange("p r s -> p (r s)"),
                        gB[:, a:b, :].rearrange("p r j -> p (r j)"),
                        start=(ck == 3 and rep == 0), stop=last_rep,
                    )

            out_sb = singles.tile([3 * SMAX, 4 * SMAX * NB], f32)
            nc.vector.memset(out_sb[:], 0.0)
            for k, cs in enumerate(BLOCK_CS):
                nc.vector.tensor_copy(
                    out=out_sb[:3 * cs, k * SMAX * NB:k * SMAX * NB + cs * NB],
                    in_=psums[k][:],
                )
            nc.sync.dma_start(out=stats_d[:], in_=out_sb[:])

    nc.finalize()
    return nc


def _get_nc():
    if "nc" not in _CACHE:
        _CACHE["nc"] = _build_nc()
    return _CACHE["nc"]


def _prep_inputs(logits, labels):
    """Shard + host-side prep. Returns in_maps for run_bass_kernel_spmd."""
    logits = np.asarray(logits)
    labels = np.asarray(labels)
    assert logits.shape == (N, C) and logits.dtype == np.float32

    bounds = np.linspace(0.0, 1.0, 16, dtype=np.float32)[:NB]
    chosen = np.take_along_axis(
        logits, labels.reshape(-1, 1).astype(np.int64), axis=1
    ).reshape(-1)

    na = T1 * P * R  # rows in the main region per core
    in_maps = []
    for c in range(NCORES):
        lo = c * ROWS_CORE
        hi = lo + ROWS_CORE
        if hi <= N:
            lg = logits[lo:hi]           # view, no copy
            ch = chosen[lo:hi]
        else:
            npad = hi - N
            lg = np.vstack([logits[lo:N], np.full((npad, C), -1.0, np.float32)])
            ch = np.concatenate([chosen[lo:N], np.zeros(npad, np.float32)])
        ch_t = np.ascontiguousarray(np.concatenate([
            ch[:na].reshape(T1, P, R).transpose(1, 0, 2).reshape(P, T1 * R),
            ch[na:].reshape(P, R2),
        ], axis=1))
        in_maps.append({"logits": lg, "chosen": ch_t, "bounds": bounds.reshape(1, NB)})
    return in_maps


def _finish(outs):
    """Fold per-core [3*SMAX, 4*SMAX*NB] stats into (ece, acc)."""
    cum_conf = np.zeros(NB, np.float64)
    cum_acc = np.zeros(NB, np.float64)
    cum_cnt = np.zeros(NB, np.float64)
    for o in outs:
        o = np.asarray(o, np.float64)
        for k, cs in enumerate(BLOCK_CS):
            off = k * SMAX * NB
            blk = o[:3 * cs, off:off + cs * NB]
            blk = blk.reshape(cs, 3, cs, NB)     # [r][s][r2][j]
            r_idx = np.arange(cs)
            cum_conf += blk[r_idx, 0, r_idx, :].sum(axis=0)
            cum_acc += blk[r_idx, 1, r_idx, :].sum(axis=0)
            cum_cnt += blk[r_idx, 2, r_idx, :].sum(axis=0)

    count = cum_cnt - np.append(cum_cnt[1:], 0.0)
    sconf = cum_conf - np.append(cum_conf[1:], 0.0)
    sacc = cum_acc - np.append(cum_acc[1:], 0.0)

    safe = count > 0
    denom = np.where(safe, count, 1.0)
    conf_in = sconf / denom
    acc_in = sacc / denom
    prop = count / float(N)
    ece = float(np.where(safe, np.abs(conf_in - acc_in) * prop, 0.0).sum() * 100.0)
    acc = float(np.where(safe, acc_in * prop, 0.0).sum() * 100.0)
    return (
        np.array([ece], np.float32),
        np.array([acc], np.float32),
    )


def _run(logits, labels, trace=False):
    from concourse.bass_utils import run_bass_kernel_spmd

    nc = _get_nc()
    in_maps = _prep_inputs(logits, labels)
    res = run_bass_kernel_spmd(
        nc, in_maps, core_ids=list(range(NCORES)), trace=trace,
    )
    outs = [r["stats"] for r in res.results]
    return _finish(outs), res


def kernel(logits, labels):
    out, _ = _run(logits, labels)
    return out
